# revision 11
# baseline (speedup 1.0000x reference)
"""ConvLSTM bottleneck block (3 stacked ConvLSTM2D layers + relu) on 8 trn2 cores.

Sharding: 8 cores = 4 batch samples x 2 horizontal half-strips (32 rows each).
Layout: channels on partitions, pixels on the free dim. Each layer keeps a
zero-padded [K, 34, 66] SBUF image holding [input; hidden] stacked on the
partition axis, so one conv (input-conv + recurrent-conv) is 9 accumulated
f32r matmuls per 512-pixel block. All three layers' h/c states stay in SBUF
across the 16 timesteps; only x comes in and relu(h3) goes out per step.
Strip boundary rows of h are exchanged between strip-pair cores once per
layer per step with a 2-rank AllGather (DRAM bounce), applied through
per-core 0/1 masks so the SPMD program is identical on all cores.

Emission is software-pipelined (L1 at t+2, L2 at t+1, L3 at t) so the PE
keeps dense matmul work while the gate chains (ACT/DVE) of earlier layers
drain.
"""
import numpy as np

import concourse.bass as bass
import concourse.mybir as mybir
import concourse.tile as tile
from concourse import bacc, bass_utils

F32 = mybir.dt.float32
F32R = mybir.dt.float32r
AF = mybir.ActivationFunctionType
OP = mybir.AluOpType

B, T, H, W, C_IN = 4, 16, 64, 64, 64
RB, CB = 34, 66          # padded strip rows / cols
NPIX = 32 * 64           # output pixels per strip per step
XFREE = RB * CB          # x-section free size

# layer geometry: (K_rows, F, n_groups)
# xh1: x@[0:64],  h1@[64:128]                      K=128
# xh2: h2@[0:32], zeros@[32:64], y1@[64:128]       K=128 (w2 zero-padded)
# xh3: h3@[0:64], y2@[64:96]                       K=96
LK = [128, 128, 96]
LF = [64, 32, 64]
LG = [2, 1, 2]

DEBUG_T0 = False


def _build(num_cores: int):
    nc = bacc.Bacc("TRN2", target_bir_lowering=False, debug=False,
                   num_devices=num_cores)
    xin = nc.dram_tensor("xin", [T, 64, XFREE], F32, kind="ExternalInput").ap()
    w1 = nc.dram_tensor("w1", [128, 9 * 256], F32, kind="ExternalInput").ap()
    w2 = nc.dram_tensor("w2", [128, 9 * 128], F32, kind="ExternalInput").ap()
    w3 = nc.dram_tensor("w3", [96, 9 * 256], F32, kind="ExternalInput").ap()
    bia = nc.dram_tensor("biases", [128, 6], F32, kind="ExternalInput").ap()
    msk = nc.dram_tensor("masks", [128, 2], F32, kind="ExternalInput").ap()
    yout = nc.dram_tensor("yout", [T, 64, NPIX], F32, kind="ExternalOutput").ap()

    groups = [[2 * i, 2 * i + 1] for i in range(max(1, num_cores // 2))]

    with tile.TileContext(nc) as tc:
        _emit(nc, tc, xin, w1, w2, w3, bia, msk, yout, groups)
    nc.compile()
    return nc


def _emit(nc, tc, xin, w1d, w2d, w3d, bia, msk, yout, groups):
    import contextlib
    ctx = contextlib.ExitStack()
    sb = ctx.enter_context(tc.tile_pool(name="sb", bufs=1))
    ps = ctx.enter_context(tc.tile_pool(name="ps", bufs=2, space="PSUM"))
    dram = ctx.enter_context(tc.tile_pool(name="dram", bufs=1, space="DRAM"))

    # persistent tiles
    w1 = sb.tile([128, 9, 256], F32R)
    w2 = sb.tile([128, 9, 128], F32R)
    w3 = sb.tile([96, 9, 256], F32R)
    biases = sb.tile([128, 6], F32)
    masks = sb.tile([128, 2], F32)
    xh1 = sb.tile([128, RB, CB], F32R)
    xh2 = sb.tile([128, RB, CB], F32R)
    xh3 = sb.tile([96, RB, CB], F32R)
    XH = [xh1, xh2, xh3]
    # gate scratch per layer (see chain emitters for slot usage)
    IF1 = sb.tile([128, NPIX], F32)
    GC1 = sb.tile([128, NPIX], F32)   # g~ @[0:64], c1 @[64:128]
    T1 = sb.tile([128, NPIX], F32)
    OO1 = sb.tile([128, NPIX], F32)
    IF3 = sb.tile([128, NPIX], F32)
    GC3 = sb.tile([128, NPIX], F32)
    T3 = sb.tile([128, NPIX], F32)
    OO3 = sb.tile([128, NPIX], F32)
    IF2 = sb.tile([64, NPIX], F32)
    BB2 = sb.tile([128, NPIX], F32)   # g~ @[64:96], o' @[96:128]
    GU2 = sb.tile([32, NPIX], F32)    # moved g~, then i*g~ (in-place)
    T2 = sb.tile([64, NPIX], F32)     # f*c @[0:32]->, tanh_c @[32:64]
    C2 = sb.tile([64, NPIX], F32)     # c2 @[32:64]

    nc.sync.dma_start(w1[:], w1d.bitcast(F32R).rearrange("p (t m) -> p t m", t=9))
    nc.sync.dma_start(w2[:], w2d.bitcast(F32R).rearrange("p (t m) -> p t m", t=9))
    nc.sync.dma_start(w3[:], w3d.bitcast(F32R).rearrange("p (t m) -> p t m", t=9))
    nc.sync.dma_start(biases[:], bia)
    nc.sync.dma_start(masks[:], msk)
    for xh in XH:
        nc.vector.memset(xh[:].bitcast(F32), 0.0)
    nc.vector.memset(GC1[:], 0.0)
    nc.vector.memset(GC3[:], 0.0)
    nc.vector.memset(C2[:], 0.0)

    WTS = [w1, w2, w3]

    def dma_x(t):
        nc.sync.dma_start(xh1[0:64, :, :], xin[t].bitcast(F32R))

    def mm_group(l, t, g):
        """emit matmuls for layer l (0-based) step t, psum group g"""
        w = WTS[l]
        xh = XH[l]
        K = LK[l]
        p = ps.tile([128, 2048], F32, tag="ps", name=f"ps_l{l}_t{t}_g{g}")
        for tap in range(9):
            ky, kx = tap // 3, tap % 3
            lhsT = w[0:K, tap, 128 * g:128 * (g + 1)]
            for b in range(4):
                rhs = xh[0:K, 8 * b + ky: 8 * b + ky + 8, kx: kx + 64]
                nc.tensor.matmul(p[:, 512 * b: 512 * (b + 1)], lhsT, rhs,
                                 start=(tap == 0), stop=(tap == 8))
        return p

    def halo(l, t):
        """exchange boundary h rows of layer l with the pair core"""
        F = LF[l]
        xh = XH[l]
        hs = [slice(64, 128), slice(0, 32), slice(0, 64)][l]
        in_b = dram.tile([F, 128], F32, tag=f"agi{l}", bufs=2,
                         name=f"agi{l}_{t}")
        out_b = dram.tile([2 * F, 128], F32, tag=f"ago{l}", bufs=2,
                          name=f"ago{l}_{t}")
        nc.sync.dma_start(in_b[:, 0:64], xh[hs, 1:2, 1:65].bitcast(F32))
        nc.sync.dma_start(in_b[:, 64:128], xh[hs, 32:33, 1:65].bitcast(F32))
        nc.gpsimd.collective_compute(
            "AllGather", OP.bypass, replica_groups=groups,
            ins=[in_b[:]], outs=[out_b[:]])
        sa = sb.tile([64, 64], F32, tag=f"sa{l}", bufs=2, name=f"sa{l}_{t}")
        sb_ = sb.tile([64, 64], F32, tag=f"sb{l}", bufs=2, name=f"sb{l}_{t}")
        nc.sync.dma_start(sa[0:F], out_b[0:F, 64:128])       # rank0 row32
        nc.sync.dma_start(sb_[0:F], out_b[F:2 * F, 0:64])    # rank1 row1
        ma = masks[0:F, 0:1]
        mb = masks[0:F, 1:2]
        # raw h into own halo rows (zero when out-of-image via mask)
        nc.vector.tensor_scalar(xh[hs, 0:1, 1:65], sa[0:F], ma, None, OP.mult)
        nc.vector.tensor_scalar(xh[hs, 33:34, 1:65], sb_[0:F], mb, None, OP.mult)
        if l < 2:
            nxt = XH[l + 1]
            ys = [slice(64, 128), slice(64, 96)][l]
            nc.vector.tensor_scalar(nxt[ys, 0:1, 1:65], sa[0:F], ma, 0.0,
                                    OP.mult, OP.max)
            nc.vector.tensor_scalar(nxt[ys, 33:34, 1:65], sb_[0:F], mb, 0.0,
                                    OP.mult, OP.max)

    def drain64_a(l, pa, IF):
        ca = 2 * l
        nc.scalar.activation(IF[:, :], pa[:, :], AF.Relu,
                             bias=biases[:, ca:ca + 1], scale=0.2)

    def drain64_b(l, pb, GC, OO):
        cb = 2 * l + 1
        nc.scalar.activation(GC[0:64], pb[0:64], AF.Tanh,
                             bias=biases[0:64, cb:cb + 1])
        nc.scalar.activation(OO[64:128], pb[64:128], AF.Relu,
                             bias=biases[64:128, cb:cb + 1], scale=0.2)

    def tail64(l, t, IF, GC, Tt, OO, ysink):
        """gate chain tail for F=64 layers (l in {0,2})"""
        nc.vector.tensor_scalar(IF[:], IF[:], 1.0, None, OP.min)
        nc.vector.tensor_scalar(OO[64:128], OO[64:128], 1.0, None, OP.min)
        nc.vector.tensor_tensor(Tt[:], IF[:], GC[:], OP.mult)  # i*g~ ; f*c
        nc.vector.tensor_copy(IF[0:64], Tt[64:128])            # reuse IF as U
        nc.vector.tensor_tensor(GC[64:128], Tt[0:64], IF[0:64], OP.add)  # c_new
        nc.scalar.activation(Tt[64:128], GC[64:128], AF.Tanh)  # tanh(c_new)
        xh = XH[l]
        hdst = xh[64:128, 1:33, 1:65] if l == 0 else xh[0:64, 1:33, 1:65]
        nc.vector.tensor_tensor(hdst, OO[64:128], Tt[64:128], OP.mult)
        # y = relu(h)
        if l == 0:
            nc.scalar.activation(XH[1][64:128, 1:33, 1:65],
                                 xh[64:128, 1:33, 1:65].bitcast(F32), AF.Relu)
        else:
            nc.scalar.activation(ysink[0:64, :],
                                 xh[0:64, 1:33, 1:65].bitcast(F32), AF.Relu)
        halo(l, t)

    def drain2(p):
        nc.scalar.activation(IF2[0:64], p[0:64], AF.Relu,
                             bias=biases[0:64, 2:3], scale=0.2)
        nc.scalar.activation(BB2[64:96], p[64:96], AF.Tanh,
                             bias=biases[64:96, 3:4])
        nc.scalar.activation(BB2[96:128], p[96:128], AF.Relu,
                             bias=biases[96:128, 3:4], scale=0.2)

    def tail2(t):
        """gate chain tail for layer 2 (F=32); psum order [i,f,g,o] by 32"""
        nc.vector.tensor_scalar(IF2[0:64], IF2[0:64], 1.0, None, OP.min)
        nc.vector.tensor_scalar(GU2[0:32], BB2[64:96], 1.0, None, OP.mult)
        nc.vector.tensor_tensor(GU2[0:32], IF2[0:32], GU2[0:32], OP.mult)  # i*g~
        nc.vector.tensor_tensor(T2[0:32], IF2[32:64], C2[32:64], OP.mult)  # f*c
        nc.vector.tensor_tensor(C2[32:64], GU2[0:32], T2[0:32], OP.add)    # c_new
        nc.scalar.activation(T2[32:64], C2[32:64], AF.Tanh)
        # o' min+move into IF2[32:64] (f already consumed)
        nc.vector.tensor_scalar(IF2[32:64], BB2[96:128], 1.0, None, OP.min)
        nc.vector.tensor_tensor(xh2[0:32, 1:33, 1:65], IF2[32:64], T2[32:64],
                                OP.mult)
        nc.scalar.activation(xh3[64:96, 1:33, 1:65],
                             xh2[0:32, 1:33, 1:65].bitcast(F32), AF.Relu)
        halo(1, t)

    # first x load; later steps are prefetched right after each MMz1 phase
    dma_x(0)

    for it in range(-2, T):
        t1, t2, t3 = it + 2, it + 1, it
        # matmul phases with psum-draining ACTs interleaved
        if 0 <= t1 < T:
            if DEBUG_T0 and t1 == 0:
                d = nc.dram_tensor("dbg_xh1", [128, RB * CB], F32,
                                   kind="ExternalOutput").ap()
                nc.sync.dma_start(d, xh1[:, :, :].bitcast(F32))
                d = nc.dram_tensor("dbg_w1", [128, 9 * 256], F32,
                                   kind="ExternalOutput").ap()
                nc.sync.dma_start(d, w1[:].bitcast(F32))
            pa1 = mm_group(0, t1, 0)
            if DEBUG_T0 and t1 == 0:
                zt = sb.tile([128, NPIX], F32, name="dbg_z_sb")
                nc.scalar.activation(zt[:], pa1[:], AF.Copy)
                d = nc.dram_tensor("dbg_z", [128, NPIX], F32,
                                   kind="ExternalOutput").ap()
                nc.sync.dma_start(d, zt[:])
            drain64_a(0, pa1, IF1)
            pb1 = mm_group(0, t1, 1)
            drain64_b(0, pb1, GC1, OO1)
            if t1 + 1 < T:
                dma_x(t1 + 1)
        if 0 <= t2 < T:
            p2 = mm_group(1, t2, 0)
            drain2(p2)
        if 0 <= t3 < T:
            pa3 = mm_group(2, t3, 0)
            drain64_a(2, pa3, IF3)
            pb3 = mm_group(2, t3, 1)
            drain64_b(2, pb3, GC3, OO3)
        # gate-chain tails (order matters: they write next-step conv inputs)
        if 0 <= t1 < T:
            tail64(0, t1, IF1, GC1, T1, OO1, None)
            if DEBUG_T0 and t1 == 0:
                dbg = {}
                for nm, tl in (("dbg_gc1", GC1), ("dbg_t1", T1),
                               ("dbg_oo1", OO1), ("dbg_if1", IF1)):
                    d = nc.dram_tensor(nm, [128, NPIX], F32,
                                       kind="ExternalOutput").ap()
                    nc.sync.dma_start(d, tl[:])
                d = nc.dram_tensor("dbg_h1", [64, NPIX], F32,
                                   kind="ExternalOutput").ap()
                nc.sync.dma_start(d, XH[0][64:128, 1:33, 1:65].bitcast(F32))
                d = nc.dram_tensor("dbg_y1", [64, NPIX], F32,
                                   kind="ExternalOutput").ap()
                nc.sync.dma_start(d, XH[1][64:128, 1:33, 1:65].bitcast(F32))
        if 0 <= t2 < T:
            tail2(t2)
        if 0 <= t3 < T:
            y_sb = sb.tile([64, NPIX], F32, tag="ysb", bufs=2,
                           name=f"ysb_{t3}")
            tail64(2, t3, IF3, GC3, T3, OO3, y_sb)
            nc.sync.dma_start(yout[t3], y_sb[0:64, :])

    ctx.close()


# ---------------------------------------------------------------------------
# host-side prep / run
# ---------------------------------------------------------------------------

_CACHED = {}


def _get_nc(num_cores=8):
    if num_cores not in _CACHED:
        _CACHED[num_cores] = _build(num_cores)
    return _CACHED[num_cores]


def _prep_weights(Wx1, Wh1, b1, Wx2, Wh2, b2, Wx3, Wh3, b3):
    w1 = np.concatenate([Wx1, Wh1], axis=2)            # [3,3,128,256]
    w1 = np.ascontiguousarray(np.transpose(w1, (2, 0, 1, 3))).reshape(128, 9 * 256)
    w2 = np.zeros((3, 3, 128, 128), np.float32)
    w2[:, :, 0:32, :] = Wh2
    w2[:, :, 64:128, :] = Wx2
    w2 = np.ascontiguousarray(np.transpose(w2, (2, 0, 1, 3))).reshape(128, 9 * 128)
    w3 = np.concatenate([Wh3, Wx3], axis=2)            # [3,3,96,256]
    w3 = np.ascontiguousarray(np.transpose(w3, (2, 0, 1, 3))).reshape(96, 9 * 256)

    def bias_cols(bvec, F):
        # returns colA [128], colB [128] aligned to psum layout
        ca = np.zeros(128, np.float32)
        cb = np.zeros(128, np.float32)
        if F == 64:
            ca[:] = 0.2 * bvec[0:128] + 0.5            # i,f
            cb[0:64] = bvec[128:192]                   # g (raw)
            cb[64:128] = 0.2 * bvec[192:256] + 0.5     # o
        else:  # F == 32, single psum group [i,f,g,o]
            ca[0:64] = 0.2 * bvec[0:64] + 0.5
            cb[64:96] = bvec[64:96]
            cb[96:128] = 0.2 * bvec[96:128] + 0.5
        return ca, cb

    cols = []
    for bvec, F in ((b1, 64), (b2, 32), (b3, 64)):
        ca, cb = bias_cols(np.asarray(bvec, np.float32), F)
        cols += [ca, cb]
    biases = np.stack(cols, axis=1)                    # [128, 6]
    return (w1.astype(np.float32), w2.astype(np.float32),
            w3.astype(np.float32), biases)


def kernel(x, Wx1, Wh1, b1, Wx2, Wh2, b2, Wx3, Wh3, b3):
    num_cores = 8
    nc = _get_nc(num_cores)
    in_maps = _make_in_maps(x, Wx1, Wh1, b1, Wx2, Wh2, b2, Wx3, Wh3, b3,
                            num_cores=num_cores)

    res = bass_utils.run_bass_kernel_spmd(
        nc, in_maps, core_ids=list(range(num_cores)))

    y = np.empty((B, T, H, W, 64), np.float32)
    for c in range(num_cores):
        b, half = c // 2, c % 2
        r0 = 32 * half
        yc = res.results[c]["yout"].reshape(T, 64, 32, 64)
        y[b, :, r0:r0 + 32, :, :] = np.transpose(yc, (0, 2, 3, 1))
    return y


def _make_in_maps(x, Wx1, Wh1, b1, Wx2, Wh2, b2, Wx3, Wh3, b3, num_cores=8):
    x = np.asarray(x, np.float32)
    w1, w2, w3, biases = _prep_weights(
        np.asarray(Wx1, np.float32), np.asarray(Wh1, np.float32), b1,
        np.asarray(Wx2, np.float32), np.asarray(Wh2, np.float32), b2,
        np.asarray(Wx3, np.float32), np.asarray(Wh3, np.float32), b3)
    in_maps = []
    for c in range(num_cores):
        b, half = c // 2, c % 2
        r0 = 32 * half
        xp = np.zeros((T, 64, RB, CB), np.float32)
        lo, hi = max(r0 - 1, 0), min(r0 + 33, 64)
        xs = x[b, :, lo:hi, :, :]
        xp[:, :, lo - (r0 - 1):hi - (r0 - 1), 1:65] = \
            np.transpose(xs, (0, 3, 1, 2))
        m = np.zeros((128, 2), np.float32)
        m[:, 0] = 1.0 if half == 1 else 0.0
        m[:, 1] = 1.0 if half == 0 else 0.0
        in_maps.append({
            "xin": xp.reshape(T, 64, XFREE),
            "w1": w1, "w2": w2, "w3": w3,
            "biases": biases, "masks": m,
        })
    return in_maps


def bench(inputs, iters=6, num_cores=8):
    """Build the sharded PJRT callable once and time repeated executions.

    Returns best-of-iters wall time in ns for one full kernel execution
    (device-side; inputs stay resident, outputs fetched but not copied).
    """
    import time
    import jax
    from jax.sharding import Mesh, PartitionSpec
    from jax.experimental.shard_map import shard_map
    from concourse import bass2jax, mybir as _mybir

    nc = _get_nc(num_cores)
    in_maps = _make_in_maps(**inputs, num_cores=num_cores)
    bass2jax.install_neuronx_cc_hook()

    partition_name = (nc.partition_id_tensor.name
                      if nc.partition_id_tensor else None)
    in_names, out_names, out_avals, zero_outs = [], [], [], []
    for alloc in nc.m.functions[0].allocations:
        if not isinstance(alloc, _mybir.MemoryLocationSet):
            continue
        name = alloc.memorylocations[0].name
        if alloc.kind == "ExternalInput":
            if name != partition_name:
                in_names.append(name)
        elif alloc.kind == "ExternalOutput":
            shape = tuple(alloc.tensor_shape)
            dtype = _mybir.dt.np(alloc.dtype)
            out_names.append(name)
            out_avals.append(jax.core.ShapedArray(shape, dtype))
            zero_outs.append(np.zeros(shape, dtype))
    n_params = len(in_names)
    all_names = in_names + out_names
    if partition_name is not None:
        all_names = all_names + [partition_name]

    def _body(*args):
        operands = list(args)
        if partition_name is not None:
            operands.append(bass2jax.partition_id_tensor())
        outs = bass2jax._bass_exec_p.bind(
            *operands, out_avals=tuple(out_avals), in_names=tuple(all_names),
            out_names=tuple(out_names), lowering_input_output_aliases=(),
            sim_require_finite=True, sim_require_nnan=True, nc=nc)
        return tuple(outs)

    devices = jax.devices()[:num_cores]
    mesh = Mesh(np.asarray(devices), ("core",))
    n_outs = len(out_names)
    sharded = jax.jit(shard_map(
        _body, mesh=mesh,
        in_specs=(PartitionSpec("core"),) * (n_params + n_outs),
        out_specs=(PartitionSpec("core"),) * n_outs, check_rep=False),
        keep_unused=True)
    concat_in = [
        np.concatenate([in_maps[c][nm] for c in range(num_cores)], axis=0)
        for nm in in_names]
    concat_zero = [np.zeros((num_cores * z.shape[0], *z.shape[1:]), z.dtype)
                   for z in zero_outs]
    args = [jax.device_put(a) for a in concat_in + concat_zero]
    out = sharded(*args)
    jax.block_until_ready(out)
    best = float("inf")
    for _ in range(iters):
        t0 = time.perf_counter()
        out = sharded(*args)
        jax.block_until_ready(out)
        best = min(best, time.perf_counter() - t0)
    return best * 1e9


# revision 21
# speedup vs baseline: 34.3633x; 34.3633x over previous
"""ConvLSTM bottleneck block (3 stacked ConvLSTM2D layers + relu) on 8 trn2 cores.

Sharding: 8 cores = 4 batch samples x 2 horizontal half-strips (32 rows each).
Layout: channels on partitions, pixels on the free dim. Each layer keeps a
zero-padded [K, 34, 66] SBUF image holding [input; hidden] stacked on the
partition axis, so one conv (input-conv + recurrent-conv) is 9 accumulated
f32r matmuls per 512-pixel block. All three layers' h/c states stay in SBUF
across the 16 timesteps; only x comes in and relu(h3) goes out per step.
Strip boundary rows of h are exchanged between strip-pair cores once per
layer per step with a 2-rank AllGather (DRAM bounce), applied through
per-core 0/1 masks so the SPMD program is identical on all cores.

Emission is software-pipelined (L1 at t+2, L2 at t+1, L3 at t) so the PE
keeps dense matmul work while the gate chains (ACT/DVE) of earlier layers
drain.
"""
import numpy as np

import concourse.bass as bass
import concourse.mybir as mybir
import concourse.tile as tile
from concourse import bacc, bass_utils

F32 = mybir.dt.float32
F32R = mybir.dt.float32r
AF = mybir.ActivationFunctionType
OP = mybir.AluOpType

B, T, H, W, C_IN = 4, 16, 64, 64, 64
RB, CB = 34, 66          # padded strip rows / cols
NPIX = 32 * 64           # output pixels per strip per step
XFREE = RB * CB          # x-section free size

# layer geometry: (K_rows, F, n_groups)
# xh1: x@[0:64],  h1@[64:128]                      K=128
# xh2: h2@[0:32], zeros@[32:64], y1@[64:128]       K=128 (w2 zero-padded)
# xh3: h3@[0:64], y2@[64:96]                       K=96
LK = [128, 128, 96]
LF = [64, 32, 64]
LG = [2, 1, 2]

PE_BF16 = False     # bf16 PE inputs (xh images + weights) instead of f32r
DEBUG_T0 = False
SKIP_AG = False     # drop the halo AllGathers (wrong at strip seam; for timing)
T_STEPS = T         # reduced-T builds for timing bisection
REPEAT = 1          # emit the whole time loop R times (timing via deltas)
HW_LOOP = 0         # wrap body in tc.For_i(R) for HW timing (needs SKIP_AG)
MM_N = 512          # matmul free dim (shrink for timing probes; wrong numerics)


def _build(num_cores: int):
    nc = bacc.Bacc("TRN2", target_bir_lowering=False, debug=False,
                   num_devices=num_cores)
    XDT = mybir.dt.bfloat16 if PE_BF16 else F32
    xin = nc.dram_tensor("xin", [T, 64, XFREE], XDT, kind="ExternalInput").ap()
    w1 = nc.dram_tensor("w1", [128, 9 * 256], XDT, kind="ExternalInput").ap()
    w2 = nc.dram_tensor("w2", [128, 9 * 128], XDT, kind="ExternalInput").ap()
    w3 = nc.dram_tensor("w3", [96, 9 * 256], XDT, kind="ExternalInput").ap()
    bia = nc.dram_tensor("biases", [128, 6], F32, kind="ExternalInput").ap()
    msk = nc.dram_tensor("masks", [128, 2], F32, kind="ExternalInput").ap()
    yout = nc.dram_tensor("yout", [T, 64, NPIX], F32, kind="ExternalOutput").ap()

    groups = [[2 * i, 2 * i + 1] for i in range(max(1, num_cores // 2))]

    with tile.TileContext(nc) as tc:
        _emit(nc, tc, xin, w1, w2, w3, bia, msk, yout, groups)
    nc.compile()
    return nc


def _emit(nc, tc, xin, w1d, w2d, w3d, bia, msk, yout, groups):
    import contextlib
    ctx = contextlib.ExitStack()
    sb = ctx.enter_context(tc.tile_pool(name="sb", bufs=1))
    ps = ctx.enter_context(tc.tile_pool(name="ps", bufs=2, space="PSUM"))
    dram = ctx.enter_context(tc.tile_pool(name="dram", bufs=1, space="DRAM"))

    # persistent tiles
    PDT = mybir.dt.bfloat16 if PE_BF16 else F32R
    w1 = sb.tile([128, 9, 256], PDT)
    w2 = sb.tile([128, 9, 128], PDT)
    w3 = sb.tile([96, 9, 256], PDT)
    biases = sb.tile([128, 6], F32)
    masks = sb.tile([128, 2], F32)
    xh1 = sb.tile([128, RB, CB], PDT)
    xh2 = sb.tile([128, RB, CB], PDT)
    xh3 = sb.tile([96, RB, CB], PDT)
    XH = [xh1, xh2, xh3]
    # gate scratch per layer (see chain emitters for slot usage)
    IF1 = sb.tile([128, NPIX], F32)
    GC1 = sb.tile([128, NPIX], F32)   # g~ @[0:64], c1 @[64:128]
    T1 = sb.tile([128, NPIX], F32)
    OO1 = sb.tile([128, NPIX], F32)
    IF3 = sb.tile([128, NPIX], F32)
    GC3 = sb.tile([128, NPIX], F32)
    T3 = sb.tile([128, NPIX], F32)
    OO3 = sb.tile([128, NPIX], F32)
    IF2 = sb.tile([64, NPIX], F32)
    BB2 = sb.tile([128, NPIX], F32)   # g~ @[64:96], o' @[96:128]
    GU2 = sb.tile([32, NPIX], F32)    # moved g~, then i*g~ (in-place)
    T2 = sb.tile([64, NPIX], F32)     # f*c @[0:32]->, tanh_c @[32:64]
    C2 = sb.tile([64, NPIX], F32)     # c2 @[32:64]

    for xh in XH:
        nc.vector.memset(xh[:] if PE_BF16 else xh[:].bitcast(F32), 0.0)
    nc.vector.memset(GC1[:], 0.0)
    nc.vector.memset(GC3[:], 0.0)
    nc.vector.memset(C2[:], 0.0)
    # x_0 and w1 gate the first matmul phase: load them first; w2/w3
    # overlap with layer-1 compute.
    nc.sync.dma_start(w1[:], w1d.bitcast(PDT).rearrange("p (t m) -> p t m", t=9))
    nc.sync.dma_start(biases[:], bia)
    nc.sync.dma_start(masks[:], msk)
    nc.gpsimd.dma_start(w2[:], w2d.bitcast(PDT).rearrange("p (t m) -> p t m", t=9))
    nc.gpsimd.dma_start(w3[:], w3d.bitcast(PDT).rearrange("p (t m) -> p t m", t=9))

    WTS = [w1, w2, w3]

    def xh_rd(ap):
        return ap if PE_BF16 else ap.bitcast(F32)

    def dma_x(t):
        nc.sync.dma_start(xh1[0:64, :, :], xin[t].bitcast(PDT))

    def mm_group(l, t, g):
        """emit matmuls for layer l (0-based) step t, psum group g"""
        w = WTS[l]
        xh = XH[l]
        K = LK[l]
        p = ps.tile([128, 2048], F32, tag="ps", name=f"ps_l{l}_t{t}_g{g}")
        nb = MM_N // 64
        for tap in range(9):
            ky, kx = tap // 3, tap % 3
            lhsT = w[0:K, tap, 128 * g:128 * (g + 1)]
            for b in range(4):
                rhs = xh[0:K, 8 * b + ky: 8 * b + ky + nb, kx: kx + 64]
                nc.tensor.matmul(p[:, 512 * b: 512 * b + MM_N], lhsT, rhs,
                                 start=(tap == 0), stop=(tap == 8))
        return p

    def halo(l, t):
        """exchange boundary h rows of layer l with the pair core"""
        if SKIP_AG:
            return
        F = LF[l]
        xh = XH[l]
        hs = [slice(64, 128), slice(0, 32), slice(0, 64)][l]
        BDT = mybir.dt.bfloat16 if PE_BF16 else F32
        in_b = dram.tile([F, 128], BDT, tag=f"agi{l}", bufs=2,
                         name=f"agi{l}_{t}")
        out_b = dram.tile([2 * F, 128], BDT, tag=f"ago{l}", bufs=2,
                          name=f"ago{l}_{t}")
        nc.sync.dma_start(in_b[:, 0:64], xh_rd(xh[hs, 1:2, 1:65]))
        nc.sync.dma_start(in_b[:, 64:128], xh_rd(xh[hs, 32:33, 1:65]))
        nc.gpsimd.collective_compute(
            "AllGather", OP.bypass, replica_groups=groups,
            ins=[in_b[:]], outs=[out_b[:]])
        sa = sb.tile([64, 64], BDT, tag=f"sa{l}", bufs=2, name=f"sa{l}_{t}")
        sb_ = sb.tile([64, 64], BDT, tag=f"sb{l}", bufs=2, name=f"sb{l}_{t}")
        nc.sync.dma_start(sa[0:F], out_b[0:F, 64:128])       # rank0 row32
        nc.sync.dma_start(sb_[0:F], out_b[F:2 * F, 0:64])    # rank1 row1
        ma = masks[0:F, 0:1]
        mb = masks[0:F, 1:2]
        # raw h into own halo rows (zero when out-of-image via mask)
        nc.vector.tensor_scalar(xh[hs, 0:1, 1:65], sa[0:F], ma, None, OP.mult)
        nc.vector.tensor_scalar(xh[hs, 33:34, 1:65], sb_[0:F], mb, None, OP.mult)
        if l < 2:
            nxt = XH[l + 1]
            ys = [slice(64, 128), slice(64, 96)][l]
            nc.vector.tensor_scalar(nxt[ys, 0:1, 1:65], sa[0:F], ma, 0.0,
                                    OP.mult, OP.max)
            nc.vector.tensor_scalar(nxt[ys, 33:34, 1:65], sb_[0:F], mb, 0.0,
                                    OP.mult, OP.max)

    def drain64_a(l, pa, IF):
        ca = 2 * l
        nc.scalar.activation(IF[:, :], pa[:, :], AF.Relu,
                             bias=biases[:, ca:ca + 1], scale=0.2)

    def drain64_b(l, pb, GC, OO):
        cb = 2 * l + 1
        nc.scalar.activation(GC[0:64], pb[0:64], AF.Tanh,
                             bias=biases[0:64, cb:cb + 1])
        nc.scalar.activation(OO[64:128], pb[64:128], AF.Relu,
                             bias=biases[64:128, cb:cb + 1], scale=0.2)

    def tail64(l, t, IF, GC, Tt, OO, ysink):
        """gate chain tail for F=64 layers (l in {0,2})"""
        nc.vector.tensor_scalar(IF[:], IF[:], 1.0, None, OP.min)
        nc.vector.tensor_scalar(OO[64:128], OO[64:128], 1.0, None, OP.min)
        nc.vector.tensor_tensor(Tt[:], IF[:], GC[:], OP.mult)  # i*g~ ; f*c
        nc.vector.tensor_copy(IF[0:64], Tt[64:128])            # reuse IF as U
        nc.vector.tensor_tensor(GC[64:128], Tt[0:64], IF[0:64], OP.add)  # c_new
        nc.scalar.activation(Tt[64:128], GC[64:128], AF.Tanh)  # tanh(c_new)
        xh = XH[l]
        hdst = xh[64:128, 1:33, 1:65] if l == 0 else xh[0:64, 1:33, 1:65]
        nc.vector.tensor_tensor(hdst, OO[64:128], Tt[64:128], OP.mult)
        # y = relu(h)
        if l == 0:
            nc.scalar.activation(XH[1][64:128, 1:33, 1:65],
                                 xh_rd(xh[64:128, 1:33, 1:65]), AF.Relu)
        else:
            nc.scalar.activation(ysink[0:64, :],
                                 xh_rd(xh[0:64, 1:33, 1:65]), AF.Relu)
        halo(l, t)

    def drain2(p):
        nc.scalar.activation(IF2[0:64], p[0:64], AF.Relu,
                             bias=biases[0:64, 2:3], scale=0.2)
        nc.scalar.activation(BB2[64:96], p[64:96], AF.Tanh,
                             bias=biases[64:96, 3:4])
        nc.scalar.activation(BB2[96:128], p[96:128], AF.Relu,
                             bias=biases[96:128, 3:4], scale=0.2)

    def tail2(t):
        """gate chain tail for layer 2 (F=32); psum order [i,f,g,o] by 32"""
        nc.vector.tensor_scalar(IF2[0:64], IF2[0:64], 1.0, None, OP.min)
        nc.vector.tensor_scalar(GU2[0:32], BB2[64:96], 1.0, None, OP.mult)
        nc.vector.tensor_tensor(GU2[0:32], IF2[0:32], GU2[0:32], OP.mult)  # i*g~
        nc.vector.tensor_tensor(T2[0:32], IF2[32:64], C2[32:64], OP.mult)  # f*c
        nc.vector.tensor_tensor(C2[32:64], GU2[0:32], T2[0:32], OP.add)    # c_new
        nc.scalar.activation(T2[32:64], C2[32:64], AF.Tanh)
        # o' min+move into IF2[32:64] (f already consumed)
        nc.vector.tensor_scalar(IF2[32:64], BB2[96:128], 1.0, None, OP.min)
        nc.vector.tensor_tensor(xh2[0:32, 1:33, 1:65], IF2[32:64], T2[32:64],
                                OP.mult)
        nc.scalar.activation(xh3[64:96, 1:33, 1:65],
                             xh_rd(xh2[0:32, 1:33, 1:65]), AF.Relu)
        halo(1, t)

    loop_cm = None
    if HW_LOOP:
        assert SKIP_AG, "collectives cannot live inside control flow"
        loop_cm = tc.For_i(0, HW_LOOP, 1)
        loop_cm.__enter__()

    # first x load; later steps are prefetched right after each MMz1 phase
    dma_x(0)

    TS = T_STEPS
    for _rep in range(REPEAT):
      for it in range(-2, TS):
          t1, t2, t3 = it + 2, it + 1, it
          # matmul phases with psum-draining ACTs interleaved
          if 0 <= t1 < TS:
                  if DEBUG_T0 and t1 == 0:
                        d = nc.dram_tensor("dbg_xh1", [128, RB * CB], F32,
                                           kind="ExternalOutput").ap()
                        nc.sync.dma_start(d, xh1[:, :, :].bitcast(F32))  # debug f32r only
                        d = nc.dram_tensor("dbg_w1", [128, 9 * 256], F32,
                                           kind="ExternalOutput").ap()
                        nc.sync.dma_start(d, w1[:].bitcast(F32))
                  pa1 = mm_group(0, t1, 0)
                  if DEBUG_T0 and t1 == 0:
                        zt = sb.tile([128, NPIX], F32, name="dbg_z_sb")
                        nc.scalar.activation(zt[:], pa1[:], AF.Copy)
                        d = nc.dram_tensor("dbg_z", [128, NPIX], F32,
                                           kind="ExternalOutput").ap()
                        nc.sync.dma_start(d, zt[:])
                  drain64_a(0, pa1, IF1)
                  pb1 = mm_group(0, t1, 1)
                  drain64_b(0, pb1, GC1, OO1)
                  if t1 + 1 < TS:
                        dma_x(t1 + 1)
          if 0 <= t2 < TS:
                  p2 = mm_group(1, t2, 0)
                  drain2(p2)
          if 0 <= t3 < TS:
                  pa3 = mm_group(2, t3, 0)
                  drain64_a(2, pa3, IF3)
                  pb3 = mm_group(2, t3, 1)
                  drain64_b(2, pb3, GC3, OO3)
          # gate-chain tails (order matters: they write next-step conv inputs)
          if 0 <= t1 < TS:
                  tail64(0, t1, IF1, GC1, T1, OO1, None)
                  if DEBUG_T0 and t1 == 0:
                        dbg = {}
                        for nm, tl in (("dbg_gc1", GC1), ("dbg_t1", T1),
                                       ("dbg_oo1", OO1), ("dbg_if1", IF1)):
                            d = nc.dram_tensor(nm, [128, NPIX], F32,
                                               kind="ExternalOutput").ap()
                            nc.sync.dma_start(d, tl[:])
                        d = nc.dram_tensor("dbg_h1", [64, NPIX], F32,
                                           kind="ExternalOutput").ap()
                        nc.sync.dma_start(d, XH[0][64:128, 1:33, 1:65].bitcast(F32))
                        d = nc.dram_tensor("dbg_y1", [64, NPIX], F32,
                                           kind="ExternalOutput").ap()
                        nc.sync.dma_start(d, XH[1][64:128, 1:33, 1:65].bitcast(F32))
          if 0 <= t2 < TS:
                  tail2(t2)
          if 0 <= t3 < TS:
                  y_sb = sb.tile([64, NPIX], F32, tag="ysb", bufs=2,
                                   name=f"ysb_{t3}")
                  tail64(2, t3, IF3, GC3, T3, OO3, y_sb)
                  nc.sync.dma_start(yout[t3], y_sb[0:64, :])

    if loop_cm is not None:
        loop_cm.__exit__(None, None, None)

    ctx.close()


# ---------------------------------------------------------------------------
# host-side prep / run
# ---------------------------------------------------------------------------

_CACHED = {}


def _get_nc(num_cores=8):
    if num_cores not in _CACHED:
        _CACHED[num_cores] = _build(num_cores)
    return _CACHED[num_cores]


def _prep_weights(Wx1, Wh1, b1, Wx2, Wh2, b2, Wx3, Wh3, b3):
    w1 = np.concatenate([Wx1, Wh1], axis=2)            # [3,3,128,256]
    w1 = np.ascontiguousarray(np.transpose(w1, (2, 0, 1, 3))).reshape(128, 9 * 256)
    w2 = np.zeros((3, 3, 128, 128), np.float32)
    w2[:, :, 0:32, :] = Wh2
    w2[:, :, 64:128, :] = Wx2
    w2 = np.ascontiguousarray(np.transpose(w2, (2, 0, 1, 3))).reshape(128, 9 * 128)
    w3 = np.concatenate([Wh3, Wx3], axis=2)            # [3,3,96,256]
    w3 = np.ascontiguousarray(np.transpose(w3, (2, 0, 1, 3))).reshape(96, 9 * 256)

    def bias_cols(bvec, F):
        # returns colA [128], colB [128] aligned to psum layout
        ca = np.zeros(128, np.float32)
        cb = np.zeros(128, np.float32)
        if F == 64:
            ca[:] = 0.2 * bvec[0:128] + 0.5            # i,f
            cb[0:64] = bvec[128:192]                   # g (raw)
            cb[64:128] = 0.2 * bvec[192:256] + 0.5     # o
        else:  # F == 32, single psum group [i,f,g,o]
            ca[0:64] = 0.2 * bvec[0:64] + 0.5
            cb[64:96] = bvec[64:96]
            cb[96:128] = 0.2 * bvec[96:128] + 0.5
        return ca, cb

    cols = []
    for bvec, F in ((b1, 64), (b2, 32), (b3, 64)):
        ca, cb = bias_cols(np.asarray(bvec, np.float32), F)
        cols += [ca, cb]
    biases = np.stack(cols, axis=1)                    # [128, 6]
    return (w1.astype(np.float32), w2.astype(np.float32),
            w3.astype(np.float32), biases)


def kernel(x, Wx1, Wh1, b1, Wx2, Wh2, b2, Wx3, Wh3, b3):
    num_cores = 8
    nc = _get_nc(num_cores)
    in_maps = _make_in_maps(x, Wx1, Wh1, b1, Wx2, Wh2, b2, Wx3, Wh3, b3,
                            num_cores=num_cores)

    res = bass_utils.run_bass_kernel_spmd(
        nc, in_maps, core_ids=list(range(num_cores)))

    y = np.empty((B, T, H, W, 64), np.float32)
    for c in range(num_cores):
        b, half = c // 2, c % 2
        r0 = 32 * half
        yc = res.results[c]["yout"].reshape(T, 64, 32, 64)
        y[b, :, r0:r0 + 32, :, :] = np.transpose(yc, (0, 2, 3, 1))
    return y


def _make_in_maps(x, Wx1, Wh1, b1, Wx2, Wh2, b2, Wx3, Wh3, b3, num_cores=8):
    x = np.asarray(x, np.float32)
    w1, w2, w3, biases = _prep_weights(
        np.asarray(Wx1, np.float32), np.asarray(Wh1, np.float32), b1,
        np.asarray(Wx2, np.float32), np.asarray(Wh2, np.float32), b2,
        np.asarray(Wx3, np.float32), np.asarray(Wh3, np.float32), b3)
    in_maps = []
    for c in range(num_cores):
        b, half = c // 2, c % 2
        r0 = 32 * half
        xp = np.zeros((T, 64, RB, CB), np.float32)
        lo, hi = max(r0 - 1, 0), min(r0 + 33, 64)
        xs = x[b, :, lo:hi, :, :]
        xp[:, :, lo - (r0 - 1):hi - (r0 - 1), 1:65] = \
            np.transpose(xs, (0, 3, 1, 2))
        m = np.zeros((128, 2), np.float32)
        m[:, 0] = 1.0 if half == 1 else 0.0
        m[:, 1] = 1.0 if half == 0 else 0.0
        in_maps.append({
            "xin": xp.reshape(T, 64, XFREE),
            "w1": w1, "w2": w2, "w3": w3,
            "biases": biases, "masks": m,
        })
    if PE_BF16:
        import ml_dtypes
        bf = ml_dtypes.bfloat16
        for m_ in in_maps:
            for k in ("xin", "w1", "w2", "w3"):
                m_[k] = m_[k].astype(bf)
    return in_maps


def bench(inputs, iters=6, num_cores=8):
    """Build the sharded PJRT callable once and time repeated executions.

    Returns best-of-iters wall time in ns for one full kernel execution
    (device-side; inputs stay resident, outputs fetched but not copied).
    """
    import time
    import jax
    from jax.sharding import Mesh, PartitionSpec
    from jax.experimental.shard_map import shard_map
    from concourse import bass2jax, mybir as _mybir

    nc = _get_nc(num_cores)
    in_maps = _make_in_maps(**inputs, num_cores=num_cores)
    bass2jax.install_neuronx_cc_hook()

    partition_name = (nc.partition_id_tensor.name
                      if nc.partition_id_tensor else None)
    in_names, out_names, out_avals, zero_outs = [], [], [], []
    for alloc in nc.m.functions[0].allocations:
        if not isinstance(alloc, _mybir.MemoryLocationSet):
            continue
        name = alloc.memorylocations[0].name
        if alloc.kind == "ExternalInput":
            if name != partition_name:
                in_names.append(name)
        elif alloc.kind == "ExternalOutput":
            shape = tuple(alloc.tensor_shape)
            dtype = _mybir.dt.np(alloc.dtype)
            out_names.append(name)
            out_avals.append(jax.core.ShapedArray(shape, dtype))
            zero_outs.append(np.zeros(shape, dtype))
    n_params = len(in_names)
    all_names = in_names + out_names
    if partition_name is not None:
        all_names = all_names + [partition_name]

    def _body(*args):
        operands = list(args)
        if partition_name is not None:
            operands.append(bass2jax.partition_id_tensor())
        outs = bass2jax._bass_exec_p.bind(
            *operands, out_avals=tuple(out_avals), in_names=tuple(all_names),
            out_names=tuple(out_names), lowering_input_output_aliases=(),
            sim_require_finite=True, sim_require_nnan=True, nc=nc)
        return tuple(outs)

    devices = jax.devices()[:num_cores]
    mesh = Mesh(np.asarray(devices), ("core",))
    n_outs = len(out_names)
    sharded = jax.jit(shard_map(
        _body, mesh=mesh,
        in_specs=(PartitionSpec("core"),) * (n_params + n_outs),
        out_specs=(PartitionSpec("core"),) * n_outs, check_rep=False),
        keep_unused=True)
    concat_in = [
        np.concatenate([in_maps[c][nm] for c in range(num_cores)], axis=0)
        for nm in in_names]
    concat_zero = [np.zeros((num_cores * z.shape[0], *z.shape[1:]), z.dtype)
                   for z in zero_outs]
    args = [jax.device_put(a) for a in concat_in + concat_zero]
    out = sharded(*args)
    jax.block_until_ready(out)
    best = float("inf")
    for _ in range(iters):
        t0 = time.perf_counter()
        out = sharded(*args)
        jax.block_until_ready(out)
        best = min(best, time.perf_counter() - t0)
    return best * 1e9
